# revision 21
# baseline (speedup 1.0000x reference)
"""AFM attention layer Trainium2 kernel.

Math: reference computes
    scores[b,i,j,h] = sum_d x[b,i,d] x[b,j,d] w[h,d] + b[h]
    s = sum_h scores ; denom[b] = sum_ij s ; out = s / denom
The head sum collapses: with wsum[d] = sum_h w[h,d], bsum = sum_h b[h]:
    S[b] = (x[b] * wsum) @ x[b]^T          (64x64, symmetric)
    denom[b] = sum_d wsum[d] * colsum[b,d]^2 + F^2 * bsum
    out[b] = (S[b] + bsum) / denom[b]
Sharding: data-parallel over batch, 512 samples per core on 8 cores.

Per-core pipeline (fp32 throughout), samples processed in pairs stacked on
the 128 SBUF partitions; engines balanced PE/ACT/GpSimd/DVE:
  1. DMA macro-tile of 32 samples as [128 (2 samples x 64 rows), 16*128]
     (SP HWDGE ring; first tile split x4 to cut the startup stall).
  2. PE transpose each pair block -> PSUM [128(d), 128(2x64 rows)].
  3. ScalarE copies PSUM -> xT (SBUF); GpSimd derives xwT = xT * wsum[p]
     (SBUF->SBUF tensor_scalar, keeps DVE free).
  4. Per sample: matmul lhsT=xwT[128,64], rhs=xT[128,64] -> S in PSUM;
     the two samples of a pair run col-tiled at tile_position (0,0)/(0,64)
     so they can execute concurrently on separate PE column groups.
  5. colsum via tiny PE matmul (lhsT = x pair block, rhs = half masks,
     N=2) interleaved between transposes so its LDWEIGHTS hides.
  6. denom: ACT Square(colsum), PE matmul vs wsum-broadcast (also
     replicates the per-sample scalar to all 128 partitions) -> DVE add
     F^2*bsum -> DVE reciprocal.
  7. (S + bsum) * inv in one DVE scalar_tensor_tensor per partition-half,
     inv broadcast along free dim with a stride-0 AP.
  8. DMA out per 8-pair block on the ACT HWDGE ring (overlaps input ring).
Built on bacc.Bacc: its compile pipeline splits multi-semaphore waits and
moves matmul waits onto LDWEIGHTS, which raw Bass+Tile output violates.
"""

import numpy as np

B, F, D = 4096, 64, 128
NCORES = 8
BS = B // NCORES            # 512 samples per core
MT_SAMPLES = 32             # samples per macro-tile
N_MT = BS // MT_SAMPLES     # 16 macro-tiles
G = MT_SAMPLES // 2         # 16 pairs per macro-tile

_CACHE = {}


DEFAULT_CFG = dict(tp=3, cp=1, sp=3, dp=1, x2=3, xt=6, xw=6, osb=4,
                   out_per_mt=False, denom_early=True)


def _build(bsum: float, cfg: dict | None = None):
    import concourse.bass as bass  # noqa: F401
    import concourse.tile as tile
    from concourse import bacc, mybir

    cfg = {**DEFAULT_CFG, **(cfg or {})}
    fp32 = mybir.dt.float32
    AF = mybir.ActivationFunctionType

    nc = bacc.Bacc("TRN2", target_bir_lowering=False, debug=False,
                   num_devices=NCORES)

    x_in = nc.declare_dram_parameter("inputs", [BS, F, D], fp32,
                                     isOutput=False)
    cst_in = nc.declare_dram_parameter("consts", [128, 259], fp32,
                                       isOutput=False)
    out_d = nc.declare_dram_parameter("out", [BS, F, F], fp32, isOutput=True)

    with tile.TileContext(nc) as tc:
        with (
            tc.tile_pool(name="cst", bufs=1) as cstp,
            tc.tile_pool(name="x2", bufs=cfg["x2"]) as xp,
            tc.tile_pool(name="xt", bufs=cfg["xt"]) as xtp,
            tc.tile_pool(name="xw", bufs=cfg["xw"]) as xwp,
            tc.tile_pool(name="csq", bufs=2) as csqp,
            tc.tile_pool(name="db", bufs=2) as dbp,
            tc.tile_pool(name="inv", bufs=2) as invp,
            tc.tile_pool(name="osb", bufs=cfg["osb"]) as op,
            tc.tile_pool(name="tps", bufs=cfg["tp"], space="PSUM") as tp,
            tc.tile_pool(name="cps", bufs=cfg["cp"], space="PSUM") as cp,
            tc.tile_pool(name="sps", bufs=cfg["sp"], space="PSUM") as sp,
            tc.tile_pool(name="dps", bufs=cfg["dp"], space="PSUM") as dp,
        ):
            cst = cstp.tile([128, 259], fp32)
            nc.scalar.dma_start(cst[:], cst_in[:])
            ident = cst[:, 0:128]
            wsumB = cst[:, 128:256]
            wcol = cst[:, 256:257]
            mask2 = cst[:, 257:259]

            def emit_denoms(C_ps):
                Csq = csqp.tile([128, MT_SAMPLES], fp32)
                nc.scalar.activation(Csq[:], C_ps[:], AF.Square)
                D_ps = dp.tile([128, MT_SAMPLES], fp32)
                nc.tensor.matmul(D_ps[:], wsumB, Csq[:],
                                 start=True, stop=True)
                Db = dbp.tile([128, MT_SAMPLES], fp32)
                nc.vector.tensor_scalar_add(
                    Db[:], D_ps[:], float(F * F) * bsum)
                inv = invp.tile([128, MT_SAMPLES], fp32)
                nc.vector.reciprocal(inv[:], Db[:])
                return inv

            for mt in range(N_MT):
                X2 = xp.tile([128, G * 128], fp32)
                nsplit = 4 if mt == 0 else 1
                step = MT_SAMPLES // nsplit
                gstep = G // nsplit
                for sp_i in range(nsplit):
                    lo = mt * MT_SAMPLES + sp_i * step
                    src = x_in[lo:lo + step].rearrange(
                        "(g two) f d -> (two f) g d", two=2)
                    nc.sync.dma_start(
                        X2[:, sp_i * gstep * 128:(sp_i + 1) * gstep * 128]
                        .rearrange("p (g d) -> p g d", g=gstep), src)

                C_ps = cp.tile([128, MT_SAMPLES], fp32)
                S_blocks = []
                inv = None
                for tb in range(G // 4):
                    T_ps = tp.tile([128, 512], fp32)
                    for k in range(4):
                        g = tb * 4 + k
                        xblk = X2[:, 128 * g:128 * (g + 1)]
                        nc.tensor.transpose(
                            T_ps[:, 128 * k:128 * (k + 1)], xblk, ident)
                        nc.tensor.matmul(
                            C_ps[:, 2 * g:2 * g + 2], xblk, mask2,
                            start=True, stop=True)
                    if tb == G // 4 - 1 and cfg["denom_early"]:
                        inv = emit_denoms(C_ps)
                    xT = xtp.tile([128, 512], fp32)
                    xwT = xwp.tile([128, 512], fp32)
                    nc.scalar.activation(xT[:], T_ps[:], AF.Copy)
                    nc.gpsimd.tensor_scalar_mul(xwT[:], xT[:], wcol)
                    if tb % 2 == 0:
                        S_ps = sp.tile([128, 512], fp32)
                        S_blocks.append(S_ps)
                    for k in range(4):
                        g = tb * 4 + k
                        c = (g % 8) * 64
                        lo = 128 * k
                        nc.tensor.matmul(
                            S_ps[0:64, c:c + 64],
                            xwT[:, lo:lo + 64], xT[:, lo:lo + 64],
                            start=True, stop=True, tile_position=(0, 0))
                        nc.tensor.matmul(
                            S_ps[64:128, c:c + 64],
                            xwT[:, lo + 64:lo + 128], xT[:, lo + 64:lo + 128],
                            start=True, stop=True, tile_position=(0, 64))

                if inv is None:
                    inv = emit_denoms(C_ps)

                if cfg["out_per_mt"]:
                    out_sb = op.tile([128, 1024], fp32)
                for sb, S_ps in enumerate(S_blocks):
                    if not cfg["out_per_mt"]:
                        out_sb = op.tile([128, 512], fp32)
                        o_lo = 0
                    else:
                        o_lo = 512 * sb
                    s0 = 16 * sb
                    inv_top = inv[0:64, s0:s0 + 16:2].broadcast_to(
                        [64, 8, 64])
                    inv_bot = inv[64:128, s0 + 1:s0 + 16:2].broadcast_to(
                        [64, 8, 64])
                    nc.vector.scalar_tensor_tensor(
                        out_sb[0:64, o_lo:o_lo + 512].rearrange(
                            "p (g j) -> p g j", j=64),
                        S_ps[0:64, :].rearrange("p (g j) -> p g j", j=64),
                        bsum, inv_top,
                        mybir.AluOpType.add, mybir.AluOpType.mult)
                    nc.vector.scalar_tensor_tensor(
                        out_sb[64:128, o_lo:o_lo + 512].rearrange(
                            "p (g j) -> p g j", j=64),
                        S_ps[64:128, :].rearrange("p (g j) -> p g j", j=64),
                        bsum, inv_bot,
                        mybir.AluOpType.add, mybir.AluOpType.mult)
                    if not cfg["out_per_mt"]:
                        dst = out_d[mt * MT_SAMPLES + s0:
                                    mt * MT_SAMPLES + s0 + 16].rearrange(
                            "(g two) f j -> (two f) g j", two=2)
                        nc.scalar.dma_start(
                            dst,
                            out_sb[:].rearrange("p (g j) -> p g j", g=8))
                if cfg["out_per_mt"]:
                    dst = out_d[mt * MT_SAMPLES:
                                (mt + 1) * MT_SAMPLES].rearrange(
                        "(g two) f j -> (two f) g j", two=2)
                    nc.scalar.dma_start(
                        dst, out_sb[:].rearrange("p (g j) -> p g j", g=16))
    nc.finalize()
    return nc


def _consts_array(wsum: np.ndarray) -> np.ndarray:
    cst = np.zeros((128, 259), dtype=np.float32)
    cst[:, 0:128] = np.eye(128, dtype=np.float32)
    cst[:, 128:256] = wsum[:, None]          # wsum along contraction axis
    cst[:, 256] = wsum                       # per-partition scalar
    cst[0:64, 257] = 1.0                     # even-sample mask
    cst[64:128, 258] = 1.0                   # odd-sample mask
    return cst


def kernel(inputs: np.ndarray, w: np.ndarray, b: np.ndarray,
           trace: bool = False, tmpdir: str | None = None):
    from concourse.bass_utils import run_bass_kernel_spmd

    inputs = np.ascontiguousarray(np.asarray(inputs, dtype=np.float32))
    w = np.asarray(w, dtype=np.float32)
    b = np.asarray(b, dtype=np.float32)
    wsum = w.sum(axis=0)
    bsum = float(b.sum())

    key = (wsum.tobytes(), bsum)
    if key not in _CACHE:
        _CACHE[key] = _build(bsum)
    nc = _CACHE[key]

    cst = _consts_array(wsum)
    shards = inputs.reshape(NCORES, BS, F, D)
    in_maps = [{"inputs": shards[i], "consts": cst} for i in range(NCORES)]
    res = run_bass_kernel_spmd(nc, in_maps, core_ids=list(range(NCORES)),
                               trace=trace, tmpdir=tmpdir)
    out = np.concatenate([r["out"] for r in res.results], axis=0)
    out = out.reshape(B, F, F).astype(np.float32)
    if trace:
        return out, res
    return out


if __name__ == "__main__":
    rng = np.random.default_rng(0)
    x = rng.standard_normal((B, F, D), dtype=np.float32)
    w = rng.standard_normal((4, D), dtype=np.float32)
    b = rng.standard_normal((4,), dtype=np.float32)
    out = kernel(x, w, b)
    wsum = w.sum(0)
    S = np.einsum('bid,bjd->bij', x * wsum, x) + b.sum()
    ref = S / S.sum(axis=(1, 2), keepdims=True)
    err = np.linalg.norm(out - ref) / np.linalg.norm(ref)
    print("rel err vs local ref:", err)
